# revision 1
# baseline (speedup 1.0000x reference)
"""CosSim attention (QKNorm + 2D image RoPE + cosine-sim softmax) on 8 trn2 cores.

Sharding: pure data-parallel over batch (b=8 -> 1 batch element per core).
Weights/tables replicated. No cross-core communication.

Per-core pipeline (matmuls in fp32r = fp22-precision full-rate PE mode):
  1. x -> xT via PE transposes                  [dm, n] layout
  2. qkv projection (per head group of 8):
     - q,k chunks -> rope (tables w/ gains folded) -> l2-normalize -> PE
       transpose -> qnT/knT [d, n] layout
     - v chunk -> v_sb [keys, head, 65] with a ones column (rowsum trick)
  3. per (head, query-chunk): S^T = knT.T @ qnT -> exp (scale=8 folded into
     ACT) -> pT; o^T/rowsum = v_aug.T @ pT; normalize via reciprocal +
     DRAM-roundtrip partition broadcast
  4. out projection from oT; DMA out

Notes:
  * nc is a Bacc (not plain Bass): Bacc.compile() splits sync waits to the
    1-per-instruction HW limit (matmuls/DMAs reject more) and moves matmul
    waits to ldweights.
  * Every fp32r matmul operand is produced by a DVE/ACT op (walrus requires
    fp32r inputs to come from a rounding producer, not plain DMA).
"""
import sys
sys.path.insert(0, '/opt/trn_rl_repo')
import numpy as np

N = 1024
DM = 1024
H = 16
D = 64
P = 128
KT = DM // P          # 8 contraction tiles
NT = N // P           # 8 token tiles
NCORES = 8
HG = 2                # head groups
HPG = H // HG         # heads per group (8)
COS_SIM_SCALE = 8.0
ROPE_THETA = 10000.0

_CACHE = {}


def _rope_tables(q_gain, k_gain):
    side = int(np.sqrt(N))
    n_freq = D // 4
    freqs = 1.0 / (ROPE_THETA ** (np.arange(n_freq, dtype=np.float64) / n_freq))
    pos = np.arange(side, dtype=np.float64)
    grid_h = np.repeat(pos, side)
    grid_w = np.tile(pos, side)
    ang = np.concatenate([grid_h[:, None] * freqs, grid_w[:, None] * freqs], axis=-1)
    ang = np.concatenate([ang, ang], axis=-1)          # (N, D)
    cos = np.cos(ang)
    sin = np.sin(ang)
    half = D // 2

    def fold(g):
        g = np.asarray(g, dtype=np.float64)
        c = (cos * g).astype(np.float32)
        s = np.empty((N, D), dtype=np.float32)
        s[:, :half] = -sin[:, :half] * g[half:]
        s[:, half:] = sin[:, half:] * g[:half]
        return c, s

    cq, sq = fold(q_gain)
    ck, sk = fold(k_gain)
    return cq, sq, ck, sk


def _build(has_bqkv, has_bout, repeat=1):
    import concourse.bass as bass
    import concourse.mybir as mybir
    import concourse.tile as tile
    from concourse import bacc
    from concourse.masks import make_identity

    f32 = mybir.dt.float32
    f32r = mybir.dt.float32r
    AF = mybir.ActivationFunctionType
    AX = mybir.AxisListType

    nc = bacc.Bacc()
    x_d = nc.dram_tensor("xt", [DM, N], f32, kind="ExternalInput")
    wqkv_d = nc.dram_tensor("wqkv", [DM, 3 * DM], f32, kind="ExternalInput")
    wout_d = nc.dram_tensor("wout", [DM, DM], f32, kind="ExternalInput")
    cosq_d = nc.dram_tensor("cosq", [N, D], f32, kind="ExternalInput")
    sinq_d = nc.dram_tensor("sinq", [N, D], f32, kind="ExternalInput")
    cosk_d = nc.dram_tensor("cosk", [N, D], f32, kind="ExternalInput")
    sink_d = nc.dram_tensor("sink", [N, D], f32, kind="ExternalInput")
    if has_bqkv:
        bqkv_d = nc.dram_tensor("bqkv", [1, 3 * DM], f32, kind="ExternalInput")
    if has_bout:
        bout_d = nc.dram_tensor("bout", [1, DM], f32, kind="ExternalInput")
    out_d = nc.dram_tensor("out", [N, DM], f32, kind="ExternalOutput")
    rcp_scr = nc.dram_tensor("rcp_scr", [2 * H * repeat, 512], f32)

    with tile.TileContext(nc) as tc:
        with (
            tc.tile_pool(name="const", bufs=1) as const,
            tc.tile_pool(name="persist", bufs=1) as persist,
            tc.tile_pool(name="wpr", bufs=2) as wpr,
            tc.tile_pool(name="stage", bufs=3) as stage,
            tc.tile_pool(name="work", bufs=2) as work,
            tc.tile_pool(name="ptp", bufs=4) as ptp,
            tc.tile_pool(name="ps_acc", bufs=2, space="PSUM") as ps_acc,
            tc.tile_pool(name="ps_s", bufs=2, space="PSUM") as ps_s,
            tc.tile_pool(name="ps_o", bufs=2, space="PSUM") as ps_o,
        ):
            onecol = const.tile([P, 1], f32)
            nc.vector.memset(onecol[:], 1.0)
            identg = const.tile([P, P], f32)
            make_identity(nc, identg[:])
            ident = const.tile([P, P], f32r)
            nc.vector.tensor_copy(ident[:], identg[:])
            if has_bqkv or has_bout:
                ones128f = const.tile([1, P], f32)
                nc.vector.memset(ones128f[:], 1.0)
                ones128 = const.tile([1, P], f32r)
                nc.vector.tensor_copy(ones128[:], ones128f[:])
            # rope tables: [128, NT, 64] (partition = token-within-tile)
            tabs = {}
            for nm, dd in (("cosq", cosq_d), ("sinq", sinq_d), ("cosk", cosk_d), ("sink", sink_d)):
                t = const.tile([P, NT, D], f32, tag="tab_" + nm)
                nc.sync.dma_start(t[:], dd[:].rearrange("(nt p) d -> p nt d", p=P))
                tabs[nm] = t

            if has_bqkv:
                bqkv = const.tile([1, 3 * DM], f32r)
                nc.gpsimd.dma_start(bqkv[:], bqkv_d[:])
            if has_bout:
                bout = const.tile([1, DM], f32r)
                nc.gpsimd.dma_start(bout[:], bout_d[:])

            def emit(rep):
                # ---- Phase 1: xT via host-pretransposed input + cast DMA ----
                xT = persist.tile([P, KT, N], f32r, tag="xT")
                nc.gpsimd.dma_start(
                    xT[:], x_d[:].rearrange("(kt p) n -> p kt n", p=P))

                oT = persist.tile([P, KT, N], f32r, tag="oT")

                # ---- Phase 2 per head group ----
                for g in range(HG):
                    qnkn = {}
                    for side in ("q", "k"):
                        cos_t = tabs["cos" + side]
                        sin_t = tabs["sin" + side]
                        col0 = (0 if side == "q" else DM) + g * 512
                        dst = persist.tile([P, 4, N], f32r, tag=side + "T")
                        qnkn[side] = dst
                        wrb = wpr.tile([P, KT, 512], f32r, tag="wr")
                        nc.gpsimd.dma_start(
                            wrb[:], wqkv_d[:, col0:col0 + 512].rearrange(
                                "(kt p) c -> p kt c", p=P))
                        wts = [wrb[:, kt, :] for kt in range(KT)]
                        for nt in range(NT):
                            acc = ps_acc.tile([P, 512], f32, tag="acc")
                            for kt in range(KT):
                                nc.tensor.matmul(
                                    acc[:], xT[:, kt, nt * P:(nt + 1) * P], wts[kt][:],
                                    start=(kt == 0),
                                    stop=(kt == KT - 1) and not has_bqkv)
                            if has_bqkv:
                                nc.tensor.matmul(
                                    acc[:], ones128[:], bqkv[:, col0:col0 + 512],
                                    start=False, stop=True)
                            # rope + l2-normalize (8 heads x 64 within the chunk)
                            q3 = acc[:].rearrange("p (h d) -> p h d", d=D)
                            tmp = work.tile([P, 512], f32, tag="tmp")
                            t3 = tmp[:].rearrange("p (h d) -> p h d", d=D)
                            sin_lo = sin_t[:, nt, 0:32][:, None, :].broadcast_to((P, HPG, 32))
                            sin_hi = sin_t[:, nt, 32:64][:, None, :].broadcast_to((P, HPG, 32))
                            cos_b = cos_t[:, nt, :][:, None, :].broadcast_to((P, HPG, D))
                            nc.vector.tensor_mul(t3[:, :, 0:32], q3[:, :, 32:64], sin_lo)
                            nc.vector.tensor_mul(t3[:, :, 32:64], q3[:, :, 0:32], sin_hi)
                            qr = work.tile([P, 512], f32, tag="qr")
                            nc.vector.tensor_mul(
                                qr[:].rearrange("p (h d) -> p h d", d=D), q3, cos_b)
                            nc.vector.tensor_add(qr[:], qr[:], tmp[:])
                            sq = work.tile([P, 512], f32, tag="tmp")
                            nc.vector.tensor_mul(sq[:], qr[:], qr[:])
                            nrm2 = work.tile([P, HPG], f32, tag="nrm2")
                            nc.vector.reduce_sum(
                                nrm2[:], sq[:].rearrange("p (h d) -> p h d", d=D),
                                axis=AX.X)
                            nrm = work.tile([P, HPG], f32, tag="nrm")
                            nc.scalar.activation(nrm[:], nrm2[:], AF.Sqrt)
                            rs = work.tile([P, HPG], f32, tag="rs")
                            nc.vector.reciprocal(rs[:], nrm[:])
                            qn = work.tile([P, 512], f32r, tag="qn")
                            nc.vector.tensor_mul(
                                qn[:].rearrange("p (h d) -> p h d", d=D),
                                qr[:].rearrange("p (h d) -> p h d", d=D),
                                rs[:, :, None].broadcast_to((P, HPG, D)))
                            tpp = ps_s.tile([P, 4, P], f32r, tag="s")
                            for st in range(4):
                                nc.tensor.transpose(
                                    tpp[:, st, :], qn[:, st * P:(st + 1) * P], ident[:])
                            nc.vector.tensor_copy(
                                dst[:, :, nt * P:(nt + 1) * P], tpp[:])

                    # --- v projection chunk ---
                    v_sb = persist.tile([P, NT, HPG, D + 1], f32r, tag="v")
                    col0 = 2 * DM + g * 512
                    wrb = wpr.tile([P, KT, 512], f32r, tag="wr")
                    nc.gpsimd.dma_start(
                        wrb[:], wqkv_d[:, col0:col0 + 512].rearrange(
                            "(kt p) c -> p kt c", p=P))
                    wts = [wrb[:, kt, :] for kt in range(KT)]
                    for nt in range(NT):
                        acc = ps_acc.tile([P, 512], f32, tag="acc")
                        for kt in range(KT):
                            nc.tensor.matmul(
                                acc[:], xT[:, kt, nt * P:(nt + 1) * P], wts[kt][:],
                                start=(kt == 0),
                                stop=(kt == KT - 1) and not has_bqkv)
                        if has_bqkv:
                            nc.tensor.matmul(
                                acc[:], ones128[:], bqkv[:, col0:col0 + 512],
                                start=False, stop=True)
                        nc.vector.tensor_copy(
                            v_sb[:, nt, :, 0:D],
                            acc[:].rearrange("p (h d) -> p h d", d=D))
                    nc.vector.tensor_copy(
                        v_sb[:, :, :, D:D + 1],
                        onecol[:, None, None, :].broadcast_to((P, NT, HPG, 1)))

                    # --- attention: head pairs (row-packed K=64 S matmuls) ---
                    qT = qnkn["q"]
                    kTt = qnkn["k"]
                    for dt in range(HPG // 2):
                        # heads 2*dt (partitions 0:64) and 2*dt+1 (64:128)
                        for qc in range(2):
                            poA = ps_o.tile([D + 1, 512], f32, tag="o")
                            poB = ps_o.tile([D + 1, 512], f32, tag="o")
                            omm = []
                            for kt in range(KT):
                                pss = ps_s.tile([P, 1024], f32, tag="s")
                                nc.tensor.matmul(
                                    pss[:, 0:512],
                                    kTt[0:D, dt, kt * P:(kt + 1) * P],
                                    qT[0:D, dt, qc * 512:(qc + 1) * 512],
                                    start=True, stop=True)
                                nc.tensor.matmul(
                                    pss[:, 512:1024],
                                    kTt[D:2 * D, dt, kt * P:(kt + 1) * P],
                                    qT[D:2 * D, dt, qc * 512:(qc + 1) * 512],
                                    start=True, stop=True)
                                pt = ptp.tile([P, 2, 512], f32r, tag="pT")
                                nc.scalar.activation(
                                    pt[:].rearrange("p a b -> p (a b)"),
                                    pss[:], AF.Exp, scale=COS_SIM_SCALE)
                                omm.append((kt, pt))
                                if len(omm) == 2:
                                    okt, opt = omm.pop(0)
                                    nc.tensor.matmul(
                                        poA[:], v_sb[:, okt, 2 * dt, :], opt[:, 0, :],
                                        start=(okt == 0), stop=False)
                                    nc.tensor.matmul(
                                        poB[:], v_sb[:, okt, 2 * dt + 1, :], opt[:, 1, :],
                                        start=(okt == 0), stop=False)
                            for okt, opt in omm:
                                nc.tensor.matmul(
                                    poA[:], v_sb[:, okt, 2 * dt, :], opt[:, 0, :],
                                    start=False, stop=(okt == KT - 1))
                                nc.tensor.matmul(
                                    poB[:], v_sb[:, okt, 2 * dt + 1, :], opt[:, 1, :],
                                    start=False, stop=(okt == KT - 1))
                            for side2, po in ((0, poA), (1, poB)):
                                hl = 2 * dt + side2
                                bp = side2 * D
                                rcp = work.tile([1, 512], f32, tag="rcp")
                                nc.vector.reciprocal(rcp[:], po[D:D + 1, :])
                                rsb = work.tile([D, 512], f32, tag="rsb")
                                scr_i = ((rep * HG + g) * HPG + hl) * 2 + qc
                                nc.sync.dma_start(rcp_scr[scr_i:scr_i + 1, :], rcp[:])
                                scr_ap = rcp_scr[scr_i:scr_i + 1, :]
                                scr_b = bass.AP(
                                    tensor=scr_ap.tensor, offset=scr_ap.offset,
                                    ap=[[0, D]] + list(scr_ap.ap)[1:])
                                nc.sync.dma_start(rsb[:], scr_b)
                                nc.vector.tensor_mul(
                                    oT[bp:bp + D, g * 4 + dt, qc * 512:(qc + 1) * 512],
                                    po[0:D, :], rsb[:])

                # ---- Phase 3: out projection ----
                for chunk in range(2):
                    wrb = wpr.tile([P, KT, 512], f32r, tag="wr")
                    nc.gpsimd.dma_start(
                        wrb[:], wout_d[:, chunk * 512:(chunk + 1) * 512].rearrange(
                            "(kt p) c -> p kt c", p=P))
                    wts = [wrb[:, kt, :] for kt in range(KT)]
                    for nt in range(NT):
                        acc = ps_acc.tile([P, 512], f32, tag="acc")
                        for kt in range(KT):
                            nc.tensor.matmul(
                                acc[:], oT[:, kt, nt * P:(nt + 1) * P], wts[kt][:],
                                start=(kt == 0),
                                stop=(kt == KT - 1) and not has_bout)
                        if has_bout:
                            nc.tensor.matmul(
                                acc[:], ones128[:], bout[:, chunk * 512:(chunk + 1) * 512],
                                start=False, stop=True)
                        ot = stage.tile([P, 512], f32, tag="ot")
                        nc.vector.tensor_copy(ot[:], acc[:])
                        nc.sync.dma_start(
                            out_d[nt * P:(nt + 1) * P, chunk * 512:(chunk + 1) * 512],
                            ot[:])

            for rep in range(repeat):
                emit(rep)

    nc.finalize()
    return nc


def kernel(x, w_qkv, b_qkv, q_gain, k_gain, w_out, b_out):
    x = np.ascontiguousarray(np.asarray(x), dtype=np.float32)
    w_qkv = np.ascontiguousarray(np.asarray(w_qkv), dtype=np.float32)
    w_out = np.ascontiguousarray(np.asarray(w_out), dtype=np.float32)
    b_qkv = np.asarray(b_qkv, dtype=np.float32)
    b_out = np.asarray(b_out, dtype=np.float32)

    has_bqkv = bool(np.any(b_qkv))
    has_bout = bool(np.any(b_out))

    key = (has_bqkv, has_bout)
    if key not in _CACHE:
        _CACHE[key] = _build(has_bqkv, has_bout)
    nc = _CACHE[key]

    cq, sq, ck, sk = _rope_tables(q_gain, k_gain)

    base = {
        "wqkv": w_qkv, "wout": w_out,
        "cosq": cq, "sinq": sq, "cosk": ck, "sink": sk,
    }
    if has_bqkv:
        base["bqkv"] = b_qkv.reshape(1, 3 * DM)
    if has_bout:
        base["bout"] = b_out.reshape(1, DM)

    in_maps = [dict(base, xt=np.ascontiguousarray(x[c].T)) for c in range(NCORES)]

    from concourse.bass_utils import run_bass_kernel_spmd
    res = run_bass_kernel_spmd(nc, in_maps, core_ids=list(range(NCORES)), trace=False)
    out = np.stack([res.results[c]["out"] for c in range(NCORES)], axis=0)
    return out.astype(np.float32)


if __name__ == "__main__":
    rng = np.random.default_rng(0)
    ins = {
        "x": rng.standard_normal((8, N, DM), dtype=np.float32),
        "w_qkv": (rng.standard_normal((DM, 3 * DM), dtype=np.float32) / 32.0),
        "b_qkv": np.zeros(3 * DM, np.float32),
        "q_gain": np.ones(D, np.float32),
        "k_gain": np.ones(D, np.float32),
        "w_out": (rng.standard_normal((DM, DM), dtype=np.float32) / 32.0),
        "b_out": np.zeros(DM, np.float32),
    }
    o = kernel(**ins)
    print(o.shape, o.dtype)



# revision 15
# speedup vs baseline: 2197.8001x; 2197.8001x over previous
"""CosSim attention (QKNorm + 2D image RoPE + cosine-sim softmax) on 8 trn2 cores.

Sharding: pure data-parallel over batch (b=8 -> 1 batch element per core).
Weights/tables replicated. No cross-core communication.

v2 vs baseline:
  * all matmuls in bf16 (2 cols/cycle PE streaming vs 1 for fp32r); fp32 PSUM
    accumulate everywhere, so only operand rounding (~0.4%) is lost -- well
    inside the 2e-2 gate (numpy emulation: 6e-3 end-to-end).
  * phase order: ALL qk-norm (Sqrt table) then ALL attention (Exp table) --
    exactly 2 LoadActFuncSet per iteration instead of 12.
  * engine rebalance: ACT does PSUM->SBUF casts + squares, Pool (gpsimd) does
    the cos-mul, DVE keeps rotate-mul/add/reduce/reciprocal/normalize.
  * rowsum reciprocal broadcast: PE K=1 ones-matmul into PSUM instead of the
    baseline's 128 DRAM round-trip DMAs.
  * qkv elementwise processed two token-tiles at a time ([128, 2, 512] PSUM
    pairs) to halve DVE/ACT instruction count.
"""
import sys
sys.path.insert(0, '/opt/trn_rl_repo')
import numpy as np

N = 1024
DM = 1024
H = 16
D = 64
P = 128
KT = DM // P          # 8 contraction tiles
NT = N // P           # 8 token tiles
NCORES = 8
HG = 2                # head groups
HPG = H // HG         # heads per group (8)
COS_SIM_SCALE = 8.0
ROPE_THETA = 10000.0

_CACHE = {}


def _rope_tables(q_gain, k_gain):
    side = int(np.sqrt(N))
    n_freq = D // 4
    freqs = 1.0 / (ROPE_THETA ** (np.arange(n_freq, dtype=np.float64) / n_freq))
    pos = np.arange(side, dtype=np.float64)
    grid_h = np.repeat(pos, side)
    grid_w = np.tile(pos, side)
    ang = np.concatenate([grid_h[:, None] * freqs, grid_w[:, None] * freqs], axis=-1)
    ang = np.concatenate([ang, ang], axis=-1)          # (N, D)
    cos = np.cos(ang)
    sin = np.sin(ang)
    half = D // 2

    def fold(g):
        g = np.asarray(g, dtype=np.float64)
        c = (cos * g).astype(np.float32)
        s = np.empty((N, D), dtype=np.float32)
        s[:, :half] = -sin[:, :half] * g[half:]
        s[:, half:] = sin[:, half:] * g[:half]
        return c, s

    cq, sq = fold(q_gain)
    ck, sk = fold(k_gain)
    return cq, sq, ck, sk


def _build(has_bqkv, has_bout, repeat=1):
    import concourse.bass as bass
    import concourse.mybir as mybir
    import concourse.tile as tile
    from concourse import bacc
    from concourse.masks import make_identity

    f32 = mybir.dt.float32
    f32r = mybir.dt.float32r
    bf16 = mybir.dt.bfloat16
    AF = mybir.ActivationFunctionType
    AX = mybir.AxisListType

    nc = bacc.Bacc()
    x_d = nc.dram_tensor("xt", [DM, N], bf16, kind="ExternalInput")
    wqkv_d = nc.dram_tensor("wqkv", [DM, 3 * DM], bf16, kind="ExternalInput")
    wout_d = nc.dram_tensor("wout", [DM, DM], bf16, kind="ExternalInput")
    cosq_d = nc.dram_tensor("cosq", [N, D], bf16, kind="ExternalInput")
    sinq_d = nc.dram_tensor("sinq", [N, D], bf16, kind="ExternalInput")
    cosk_d = nc.dram_tensor("cosk", [N, D], bf16, kind="ExternalInput")
    sink_d = nc.dram_tensor("sink", [N, D], bf16, kind="ExternalInput")
    if has_bqkv:
        bqkv_d = nc.dram_tensor("bqkv", [1, 3 * DM], f32, kind="ExternalInput")
    if has_bout:
        bout_d = nc.dram_tensor("bout", [1, DM], f32, kind="ExternalInput")
    out_d = nc.dram_tensor("out", [N, DM], f32, kind="ExternalOutput")

    NT2 = NT // 2  # token-tile pairs

    with tile.TileContext(nc) as tc:
        with (
            tc.tile_pool(name="const", bufs=1) as const,
            tc.tile_pool(name="persist", bufs=1) as persist,
            tc.tile_pool(name="wpr", bufs=2) as wpr,
            tc.tile_pool(name="stage", bufs=3) as stage,
            tc.tile_pool(name="work", bufs=3) as work,
            tc.tile_pool(name="ptp", bufs=8) as ptp,
            tc.tile_pool(name="ps_acc", bufs=2, space="PSUM") as ps_acc,   # 2x1 bank
            tc.tile_pool(name="ps_s", bufs=2, space="PSUM") as ps_s,       # 2x2 banks
            tc.tile_pool(name="ps_o", bufs=1, space="PSUM") as ps_o,       # 1x2 banks
        ):
            onecol = const.tile([P, 1], bf16)
            nc.vector.memset(onecol[:], 1.0)
            identg = const.tile([P, P], f32)
            make_identity(nc, identg[:])
            ident = const.tile([P, P], bf16)
            nc.vector.tensor_copy(ident[:], identg[:])
            if has_bqkv or has_bout:
                ones128f = const.tile([1, P], f32)
                nc.vector.memset(ones128f[:], 1.0)
                ones128 = const.tile([1, P], bf16)
                nc.vector.tensor_copy(ones128[:], ones128f[:])
            # rope tables: [128, NT, 64] (partition = token-within-tile)
            tabs = {}
            for nm, dd in (("cosq", cosq_d), ("sinq", sinq_d), ("cosk", cosk_d), ("sink", sink_d)):
                t = const.tile([P, NT, D], bf16, tag="tab_" + nm)
                nc.scalar.dma_start(t[:], dd[:].rearrange("(nt p) d -> p nt d", p=P))
                tabs[nm] = t

            if has_bqkv:
                bqkvf = const.tile([1, 3 * DM], f32)
                nc.scalar.dma_start(bqkvf[:], bqkv_d[:])
                bqkv = const.tile([1, 3 * DM], bf16)
                nc.vector.tensor_copy(bqkv[:], bqkvf[:])
            if has_bout:
                boutf = const.tile([1, DM], f32)
                nc.scalar.dma_start(boutf[:], bout_d[:])
                bout = const.tile([1, DM], bf16)
                nc.vector.tensor_copy(bout[:], boutf[:])

            def emit(rep):
                # ---- Phase 1: xT via host-pretransposed bf16 input ----
                xr = x_d[:].rearrange("(kt p) n -> p kt n", p=P)
                xTs = []
                for kt in range(KT):
                    t = persist.tile([P, N], bf16, tag=f"xT{kt}")
                    nc.sync.dma_start(t[:], xr[:, kt, :])
                    xTs.append(t)

                oT = persist.tile([P, KT, N], bf16, tag="oT")

                qk = {}   # (g, side) -> [P, 4, N] bf16, transposed layout
                vs = {}   # g -> [P, NT, HPG, D+1] bf16

                # ---- Phase 2: qkv projection + rope + l2-norm (Sqrt epoch) ----
                # Transposes for tile i are emitted after tile i+1's matmuls
                # (pending_tp) so the PE never waits on the rope chain.
                pending_tp = []

                def flush_tp(keep=0):
                    while len(pending_tp) > keep:
                        pending_tp.pop(0)()

                for g in range(HG):
                    for side in ("q", "k"):
                        sin_t = tabs["sin" + side]
                        col0 = (0 if side == "q" else DM) + g * 512
                        dst = persist.tile([P, 4, N], bf16, tag=f"{side}T{g}")
                        qk[(g, side)] = dst
                        wsrc = wqkv_d[:, col0:col0 + 512].rearrange(
                            "(kt p) c -> p kt c", p=P)
                        wts = []
                        for kt in range(KT):
                            wt = wpr.tile([P, 512], bf16, tag=f"wr{kt}")
                            nc.scalar.dma_start(wt[:], wsrc[:, kt, :])
                            wts.append(wt)
                        for nt2 in range(NT2):
                            qb = work.tile([P, 2, 512], bf16, tag="qb")
                            for half in range(2):
                                nt = nt2 * 2 + half
                                acc = ps_acc.tile([P, 512], f32, tag="acc")
                                for kt in range(KT):
                                    nc.tensor.matmul(
                                        acc[:],
                                        xTs[kt][:, nt * P:(nt + 1) * P], wts[kt][:],
                                        start=(kt == 0),
                                        stop=(kt == KT - 1) and not has_bqkv)
                                if has_bqkv:
                                    nc.tensor.matmul(
                                        acc[:], ones128[:],
                                        bqkv[:, col0:col0 + 512],
                                        start=False, stop=True)
                                # PSUM -> SBUF bf16 cast on ACT
                                nc.scalar.copy(qb[:, half, :], acc[:])
                            flush_tp(keep=1)
                            q3 = qb[:].rearrange("p a (h d) -> p (a h) d", d=D)
                            # rope: t3 = rotate_half(q) * sin  (2 DVE ops)
                            tmp = work.tile([P, 2, 512], bf16, tag="tmp")
                            t3 = tmp[:].rearrange("p a (h d) -> p (a h) d", d=D)
                            for half in range(2):
                                nt = nt2 * 2 + half
                                hsl = slice(half * HPG, (half + 1) * HPG)
                                nc.vector.tensor_mul(
                                    t3[:, hsl, 0:32], q3[:, hsl, 32:64],
                                    sin_t[:, nt, 0:32][:, None, :].broadcast_to((P, HPG, 32)))
                                nc.vector.tensor_mul(
                                    t3[:, hsl, 32:64], q3[:, hsl, 0:32],
                                    sin_t[:, nt, 32:64][:, None, :].broadcast_to((P, HPG, 32)))
                            # qc = q * cos on Pool
                            qcm = work.tile([P, 2, 512], bf16, tag="qcm")
                            c3 = qcm[:].rearrange("p a (h d) -> p a h d", d=D)
                            nc.vector.tensor_mul(
                                c3,
                                qb[:].rearrange("p a (h d) -> p a h d", d=D),
                                tabs["cos" + side][:, nt2 * 2:nt2 * 2 + 2, :][:, :, None, :].broadcast_to((P, 2, HPG, D)))
                            qr = work.tile([P, 2, 512], bf16, tag="qr")
                            nc.vector.tensor_add(qr[:], qcm[:], tmp[:])
                            # sum of squares per head (ACT square + DVE reduce)
                            sq = work.tile([P, 2, 512], bf16, tag="tmp")
                            nc.scalar.square(sq[:], qr[:])
                            nrm2 = work.tile([P, 2 * HPG], f32, tag="nrm2")
                            nc.vector.reduce_sum(
                                nrm2[:], sq[:].rearrange("p a (h d) -> p (a h) d", d=D),
                                axis=AX.X)
                            nrm = work.tile([P, 2 * HPG], f32, tag="nrm")
                            nc.scalar.sqrt(nrm[:], nrm2[:])
                            rs = work.tile([P, 2 * HPG], f32, tag="rs")
                            nc.vector.reciprocal(rs[:], nrm[:])
                            qn = work.tile([P, 2, 512], bf16, tag="qn")
                            nc.vector.tensor_mul(
                                qn[:].rearrange("p a (h d) -> p (a h) d", d=D),
                                qr[:].rearrange("p a (h d) -> p (a h) d", d=D),
                                rs[:, :, None].broadcast_to((P, 2 * HPG, D)))

                            def make_tp(qn=qn, dst=dst, nt2=nt2):
                                def tp():
                                    # transpose to [d, tokens] layout (PE)
                                    tpp = ps_s.tile([P, 8, P], bf16, tag="s")
                                    for half in range(2):
                                        for st in range(4):
                                            nc.tensor.transpose(
                                                tpp[:, half * 4 + st, :],
                                                qn[:, half, st * P:(st + 1) * P],
                                                ident[:])
                                    nc.vector.tensor_copy(
                                        dst[:, :, nt2 * 2 * P:(nt2 * 2 + 2) * P].rearrange(
                                            "p s (a q) -> p s a q", a=2),
                                        tpp[:].rearrange("p (a s) q -> p s a q", a=2))
                                return tp
                            pending_tp.append(make_tp())

                    # --- v projection chunk ---
                    v_sb = persist.tile([P, NT, HPG, D + 1], bf16, tag=f"v{g}")
                    vs[g] = v_sb
                    col0 = 2 * DM + g * 512
                    wsrc = wqkv_d[:, col0:col0 + 512].rearrange(
                        "(kt p) c -> p kt c", p=P)
                    wts = []
                    for kt in range(KT):
                        wt = wpr.tile([P, 512], bf16, tag=f"wr{kt}")
                        nc.scalar.dma_start(wt[:], wsrc[:, kt, :])
                        wts.append(wt)
                    for nt in range(NT):
                        acc = ps_acc.tile([P, 512], f32, tag="acc")
                        for kt in range(KT):
                            nc.tensor.matmul(
                                acc[:],
                                xTs[kt][:, nt * P:(nt + 1) * P], wts[kt][:],
                                start=(kt == 0),
                                stop=(kt == KT - 1) and not has_bqkv)
                        if has_bqkv:
                            nc.tensor.matmul(
                                acc[:], ones128[:],
                                bqkv[:, col0:col0 + 512],
                                start=False, stop=True)
                        flush_tp()
                        nc.vector.tensor_copy(
                            v_sb[:, nt, :, 0:D],
                            acc[:].rearrange("p (h d) -> p h d", d=D))
                    nc.vector.tensor_copy(
                        v_sb[:, :, :, D:D + 1],
                        onecol[:, None, None, :].broadcast_to((P, NT, HPG, 1)))

                flush_tp()

                # ---- Phase 3: attention (Exp epoch) ----
                pending_ep = []

                def flush_ep():
                    while pending_ep:
                        pending_ep.pop(0)()

                for g in range(HG):
                    qT = qk[(g, "q")]
                    kTt = qk[(g, "k")]
                    v_sb = vs[g]
                    for dt in range(HPG // 2):
                        for qc in range(2):
                            po = ps_o.tile([D + 1, 2, 512], f32, tag="po")
                            omm = []
                            for kt in range(KT):
                                if kt == 2:
                                    flush_ep()
                                pss = ps_s.tile([P, 1024], f32, tag="s")
                                nc.tensor.matmul(
                                    pss[:, 0:512],
                                    kTt[0:D, dt, kt * P:(kt + 1) * P],
                                    qT[0:D, dt, qc * 512:(qc + 1) * 512],
                                    start=True, stop=True)
                                nc.tensor.matmul(
                                    pss[:, 512:1024],
                                    kTt[D:2 * D, dt, kt * P:(kt + 1) * P],
                                    qT[D:2 * D, dt, qc * 512:(qc + 1) * 512],
                                    start=True, stop=True)
                                pt = ptp.tile([P, 2, 512], bf16, tag="pT")
                                nc.scalar.activation(
                                    pt[:].rearrange("p a b -> p (a b)"),
                                    pss[:], AF.Exp, scale=COS_SIM_SCALE)
                                omm.append((kt, pt))
                                if len(omm) == 6:
                                    okt, opt = omm.pop(0)
                                    nc.tensor.matmul(
                                        po[:, 0, :], v_sb[:, okt, 2 * dt, :],
                                        opt[:, 0, :],
                                        start=(okt == 0), stop=False)
                                    nc.tensor.matmul(
                                        po[:, 1, :], v_sb[:, okt, 2 * dt + 1, :],
                                        opt[:, 1, :],
                                        start=(okt == 0), stop=False)
                            for okt, opt in omm:
                                nc.tensor.matmul(
                                    po[:, 0, :], v_sb[:, okt, 2 * dt, :],
                                    opt[:, 0, :],
                                    start=False, stop=(okt == KT - 1))
                                nc.tensor.matmul(
                                    po[:, 1, :], v_sb[:, okt, 2 * dt + 1, :],
                                    opt[:, 1, :],
                                    start=False, stop=(okt == KT - 1))
                            # normalize: rcp = 1/rowsum (DVE), partition-
                            # broadcast on Pool (SBUF->SBUF), then muls that
                            # read only one PSUM operand. Deferred into the
                            # next (dt,qc) iteration's S-matmul stream so the
                            # PE queue never blocks on the reciprocal chain.
                            rcp = work.tile([1, 2, 512], f32, tag="rcp")
                            nc.vector.reciprocal(rcp[:], po[D:D + 1, :, :])
                            brs = [work.tile([D, 512], f32, tag=f"br{s2}",
                                             name=f"br{s2}")
                                   for s2 in range(2)]
                            for side2 in range(2):
                                nc.gpsimd.partition_broadcast(
                                    brs[side2][:], rcp[:, side2, :])

                            def make_ep(po=po, brs=brs, g=g, dt=dt, qc=qc):
                                def ep():
                                    for side2 in range(2):
                                        bp = side2 * D
                                        nc.vector.tensor_mul(
                                            oT[bp:bp + D, g * 4 + dt,
                                               qc * 512:(qc + 1) * 512],
                                            po[0:D, side2, :], brs[side2][:])
                                return ep
                            pending_ep.append(make_ep())

                flush_ep()

                # ---- Phase 4: out projection ----
                for chunk in range(2):
                    wsrc = wout_d[:, chunk * 512:(chunk + 1) * 512].rearrange(
                        "(kt p) c -> p kt c", p=P)
                    wts = []
                    for kt in range(KT):
                        wt = wpr.tile([P, 512], bf16, tag=f"wr{kt}")
                        nc.scalar.dma_start(wt[:], wsrc[:, kt, :])
                        wts.append(wt)
                    for nt in range(NT):
                        acc = ps_acc.tile([P, 512], f32, tag="acc")
                        for kt in range(KT):
                            nc.tensor.matmul(
                                acc[:], oT[:, kt, nt * P:(nt + 1) * P], wts[kt][:],
                                start=(kt == 0),
                                stop=(kt == KT - 1) and not has_bout)
                        if has_bout:
                            nc.tensor.matmul(
                                acc[:], ones128[:],
                                bout[:, chunk * 512:(chunk + 1) * 512],
                                start=False, stop=True)
                        ot = stage.tile([P, 512], f32, tag="ot")
                        nc.scalar.copy(ot[:], acc[:])
                        nc.sync.dma_start(
                            out_d[nt * P:(nt + 1) * P, chunk * 512:(chunk + 1) * 512],
                            ot[:])

            for rep in range(repeat):
                emit(rep)

    nc.finalize()
    return nc


def _make_in_maps(x, w_qkv, b_qkv, q_gain, k_gain, w_out, b_out):
    """Host-side input prep: transpose/cast to the kernel's DRAM layout."""
    import ml_dtypes
    bf16 = ml_dtypes.bfloat16

    x = np.asarray(x, dtype=np.float32)
    w_qkv = np.asarray(w_qkv, dtype=np.float32)
    w_out = np.asarray(w_out, dtype=np.float32)
    b_qkv = np.asarray(b_qkv, dtype=np.float32)
    b_out = np.asarray(b_out, dtype=np.float32)

    has_bqkv = bool(np.any(b_qkv))
    has_bout = bool(np.any(b_out))

    cq, sq, ck, sk = _rope_tables(q_gain, k_gain)
    base = {
        "wqkv": np.ascontiguousarray(w_qkv).astype(bf16),
        "wout": np.ascontiguousarray(w_out).astype(bf16),
        "cosq": cq.astype(bf16), "sinq": sq.astype(bf16),
        "cosk": ck.astype(bf16), "sink": sk.astype(bf16),
    }
    if has_bqkv:
        base["bqkv"] = b_qkv.reshape(1, 3 * DM)
    if has_bout:
        base["bout"] = b_out.reshape(1, DM)
    in_maps = [dict(base, xt=np.ascontiguousarray(x[c].T).astype(bf16))
               for c in range(NCORES)]
    return in_maps, has_bqkv, has_bout


def kernel(x, w_qkv, b_qkv, q_gain, k_gain, w_out, b_out):
    in_maps, has_bqkv, has_bout = _make_in_maps(
        x, w_qkv, b_qkv, q_gain, k_gain, w_out, b_out)

    key = (has_bqkv, has_bout)
    if key not in _CACHE:
        _CACHE[key] = _build(has_bqkv, has_bout)
    nc = _CACHE[key]

    from concourse.bass_utils import run_bass_kernel_spmd
    res = run_bass_kernel_spmd(nc, in_maps, core_ids=list(range(NCORES)), trace=False)
    out = np.stack([res.results[c]["out"] for c in range(NCORES)], axis=0)
    return out.astype(np.float32)


if __name__ == "__main__":
    rng = np.random.default_rng(0)
    ins = {
        "x": rng.standard_normal((8, N, DM), dtype=np.float32),
        "w_qkv": (rng.standard_normal((DM, 3 * DM), dtype=np.float32) / 32.0),
        "b_qkv": np.zeros(3 * DM, np.float32),
        "q_gain": np.ones(D, np.float32),
        "k_gain": np.ones(D, np.float32),
        "w_out": (rng.standard_normal((DM, DM), dtype=np.float32) / 32.0),
        "b_out": np.zeros(DM, np.float32),
    }
    o = kernel(**ins)
    print(o.shape, o.dtype)
